# revision 19
# baseline (speedup 1.0000x reference)
"""Trainium2 Bass kernel for nn_Decoder_72928544686482.

Pointer-gen style decoder: outer scan of T-1=31 steps, each running a
32-step LSTM (B=64, E=256, U=512, gates=2048) => 992 sequential LSTM
cell steps, plus per-outer-step input projection, p_gen dense,
argmax -> embedding feedback.

Strategy: data-parallel over batch B across 8 cores (8 rows/core), no
inter-core communication. Everything on-chip in "FLIP" layout
([feature, (t, b)]) so VectorE/ScalarE always work on 128-partition
tiles. The recurrent h @ Wr runs as weight-stationary fused matmuls
accumulating onto PSUM-preloaded input projections (xg).
"""

import os
import sys

import numpy as np

sys.path.insert(0, "/opt/trn_rl_repo")

import concourse.mybir as mybir
from concourse import bacc, bass
from concourse.bass_utils import run_bass_kernel_spmd
from concourse.tile import TileContext

E, U, T, B, NC = 256, 512, 32, 64, 8
BS = B // NC          # 8 batch rows per core
G = 4 * U             # 2048 gate columns
TB = T * BS           # 256 = (t, b) flattened, t-major
KO = T - 1            # 31 outer steps
SEG_MAX = 16          # outer steps per NEFF launch (sem-counter headroom)
NKU = U // 128        # 4 u-chunks
NM = G // 128         # 16 gate m-tiles

# packed-input blob layout: name -> (offset, length) in fp32 elems per partition
_BL = [("wf", 7 * G), ("dec", 3 * TB), ("h0", NKU * BS), ("c0", NKU * BS),
       ("embl", NKU * E), ("wgh", NKU * U), ("wgd", 3 * U), ("wge", NKU * U),
       ("encT", NKU * TB), ("ident", 128), ("onesc", 128)]
BLOB_OFF = {}
_o = 0
for _n, _l in _BL:
    BLOB_OFF[_n] = (_o, _l)
    _o += _l
BLOB_N = _o
F32 = mybir.dt.float32
F32R = mybir.dt.float32r
AF = mybir.ActivationFunctionType
ALU = mybir.AluOpType
AX = mybir.AxisListType
# transpose-mode matmuls: fp32 weight path is reportedly faster; A/B via env
TMODE = os.environ.get("TMODE", "0") == "1"
TM = True if TMODE else None


def build(n_outer=KO, rec_r=False, big_r=False, carry_out=False):
    """Build the Bass program (shared by all 8 cores).

    rec_r / big_r: use float32r (same bits as fp32, faster PE streaming
    mode) for the recurrence matmuls / the N=256 matmuls.
    """
    nc = bacc.Bacc(None, target_bir_lowering=False)

    def dp(name, shape, out=False):
        return nc.declare_dram_parameter(name, list(shape), F32, isOutput=out)

    # All inputs packed into ONE dram tensor => one DMA => one DMAHW sem
    # lane (consumers can only encode a few wait conditions).
    blob_d = dp("blob", [128, BLOB_N])

    odo_d = dp("odo", [n_outer, 128, NKU, TB], out=True)   # dec_outs FLIP
    opg_d = dp("opg", [n_outer, 128, NKU, TB], out=True)   # p_gens FLIP
    ohs_d = dp("ohs", [n_outer, 128, NKU, BS], out=True)
    ocs_d = dp("ocs", [n_outer, 128, NKU, BS], out=True)
    if carry_out:
        ocd_d = dp("ocd", [128, 3, TB], out=True)
        och_d = dp("och", [128, NKU, BS], out=True)
        occ_d = dp("occ", [128, NKU, BS], out=True)

    def mR(ap):
        return ap.bitcast(F32R) if rec_r else ap

    def mB(ap):
        return ap.bitcast(F32R) if big_r else ap

    with TileContext(nc) as tc:
        with (
            tc.tile_pool(name="w", bufs=1) as wp,
            tc.tile_pool(name="ps", bufs=1, space="PSUM") as pp,
        ):
            # ---- persistent SBUF: one blob holding every input ----
            blob = wp.tile([128, BLOB_N], F32, tag="blob")
            nc.sync.dma_start(out=blob[:, :], in_=blob_d[:, :])

            def bv(name, shape):
                o, n = BLOB_OFF[name]
                ap = blob[:, o:o + n]
                if len(shape) == 3:
                    ap = ap.rearrange("p (a b) -> p a b", a=shape[1])
                return ap

            wf = bv("wf", [128, 7, G])
            dec = bv("dec", [128, 3, TB])
            h0 = bv("h0", [128, NKU, BS])
            c_a = bv("c0", [128, NKU, BS])
            c_b = wp.tile([128, NKU, BS], F32, tag="c_b")
            embl = bv("embl", [128, NKU, E])
            wgh = bv("wgh", [128, NKU, U])
            wgd = bv("wgd", [128, 3, U])
            wge = bv("wge", [128, NKU, U])
            encT = bv("encT", [128, NKU, TB])
            ident = bv("ident", [128, 128])
            onesc = bv("onesc", [128, 128])[0:1, :]

            enc_pre = wp.tile([128, NKU, TB], F32, tag="enc_pre")
            outs_ab = [
                wp.tile([128, NKU, TB], F32, tag="outsA", name="outsA"),
                wp.tile([128, NKU, TB], F32, tag="outsB", name="outsB"),
            ]
            onehot = wp.tile([128, NKU, TB], F32, tag="onehot")
            maxrow = wp.tile([1, TB], F32, tag="maxrow")
            maxcol = wp.tile([128, 2], F32, tag="maxcol")
            pgf = wp.tile([128, NKU, TB], F32, tag="pgf")
            s_if = wp.tile([128, 8, BS], F32, tag="s_if")
            s_cc = wp.tile([128, NKU, BS], F32, tag="s_cc")
            s_o = wp.tile([128, NKU, BS], F32, tag="s_o")
            s_tc = wp.tile([128, NKU, BS], F32, tag="s_tc")
            tmp_ic = wp.tile([128, NKU, BS], F32, tag="tmp_ic")
            ctmp = wp.tile([128, NKU, BS], F32, tag="ctmp")
            zeros8 = wp.tile([128, 8, T, BS], F32, tag="zeros8")
            nc.vector.memset(zeros8[:, :, :, :], 0.0)

            # Whole PSUM as one tile, viewed [128, m(16), t(32), b(8)].
            # bank b = ps[:, 2b:2b+2, :, :]
            ps = pp.tile([128, NM, T, BS], F32, tag="ps")

            # Input DMAs fan out across many HW-DGE queues; a consumer
            # matmul cannot encode that many waits. Fence once here.
            tc.strict_bb_all_engine_barrier()

            # ---- startup: enc_pre = enc^T-part of p_gen, via banks 0-3 ----
            for mo in range(4):
                for kk in range(NKU):
                    nc.tensor.matmul(
                        ps[:, 2 * mo, :, :],
                        mB(wge[:, kk, mo * 128:(mo + 1) * 128]),
                        mB(encT[:, kk, :]),
                        start=(kk == 0), stop=(kk == NKU - 1),
                        skip_group_check=True,
                    )
                nc.scalar.copy(enc_pre[:, mo, :], ps[:, 2 * mo, :, :].rearrange("p t b -> p (t b)"))

            # ---- xg preload (+ bias row via const ones chunk) ----
            # PSUM has_written semantics make intra-bank start=True unsafe
            # (start clears the *whole bank's* bits and each bank holds two
            # m-tiles), so: memset data to zero and use start=False
            # everywhere — each element is then either overwritten (bit
            # clear) or added to zero (bit set); both are correct.
            def xg_load():
                nc.vector.memset(ps[:, 0:8, :, :], 0.0)
                nc.scalar.copy(ps[:, 8:16, :, :], zeros8[:, :, :, :])
                for m in range(NM):
                    for ee in range(3):
                        nc.tensor.matmul(
                            ps[:, m, :, :],
                            mB(wf[:, 4 + ee, m * 128:(m + 1) * 128]),
                            mB(dec[:, ee, :]),
                            is_transpose=TM,
                            start=False, stop=False,
                            skip_group_check=True,
                        )

            # Collapse the startup DMA/compute dependency fan-in into one
            # barrier — walrus rejects matmuls with too many sync waits.
            tc.strict_bb_all_engine_barrier()

            xg_load()

            c_prev = c_a
            c_pool = [c_a, c_b]
            h_pre = h0[:, :, :]

            for k in range(n_outer):
                outs = outs_ab[k % 2]
                # outputs of the *pre-scan* state
                nc.sync.dma_start(out=ohs_d[k], in_=h_pre)
                nc.sync.dma_start(out=ocs_d[k], in_=c_prev[:, :, :])

                # ---- 32 LSTM cell steps ----
                for t in range(T):
                    h_src = h_pre if t == 0 else outs[:, :, (t - 1) * BS:t * BS]
                    for m in range(NM):
                        for kk in range(NKU):
                            nc.tensor.matmul(
                                ps[:, m, t, :],
                                mR(wf[:, kk, m * 128:(m + 1) * 128]),
                                mR(h_src[:, kk, :]),
                                is_transpose=TM,
                                start=False, stop=(kk == NKU - 1),
                                skip_group_check=True,
                            )
                    nc.scalar.activation(s_if[:, :, :], ps[:, 0:8, t, :], AF.Sigmoid)
                    nc.scalar.activation(s_cc[:, :, :], ps[:, 8:12, t, :], AF.Tanh)
                    nc.scalar.activation(s_o[:, :, :], ps[:, 12:16, t, :], AF.Sigmoid)
                    nc.vector.tensor_tensor(
                        out=tmp_ic[:, :, :], in0=s_if[:, 0:4, :], in1=s_cc[:, :, :],
                        op=ALU.mult)
                    c_next = c_pool[(t + 1) % 2]
                    nc.vector.tensor_tensor(
                        out=ctmp[:, :, :], in0=s_if[:, 4:8, :], in1=c_prev[:, :, :],
                        op=ALU.mult)
                    nc.vector.tensor_tensor(
                        out=c_next[:, :, :], in0=ctmp[:, :, :], in1=tmp_ic[:, :, :],
                        op=ALU.add)
                    nc.scalar.activation(s_tc[:, :, :], c_next[:, :, :], AF.Tanh)
                    nc.vector.tensor_tensor(
                        out=outs[:, :, t * BS:(t + 1) * BS], in0=s_o[:, :, :],
                        in1=s_tc[:, :, :], op=ALU.mult)
                    c_prev = c_next

                nc.sync.dma_start(out=odo_d[k], in_=outs[:, :, :])

                # ---- p_gen (pre-scan h, current dec_in, static enc part) ----
                for mo in range(4):
                    for ee in range(3):
                        nc.tensor.matmul(
                            ps[:, 2 * mo, :, :],
                            mB(wgd[:, ee, mo * 128:(mo + 1) * 128]),
                            mB(dec[:, ee, :]),
                            start=(ee == 0), stop=(ee == 2),
                            skip_group_check=True,
                        )
                for mo in range(4):
                    for kk in range(NKU):
                        nc.tensor.matmul(
                            ps[:, 8, mo, :],
                            mR(wgh[:, kk, mo * 128:(mo + 1) * 128]),
                            mR(h_pre[:, kk, :]),
                            start=(kk == 0), stop=(kk == NKU - 1),
                            skip_group_check=True,
                        )
                for mo in range(4):
                    pgm = pgf[:, mo, :].rearrange("p (t b) -> p t b", t=T)
                    nc.vector.tensor_tensor(
                        out=pgm, in0=ps[:, 2 * mo, :, :],
                        in1=enc_pre[:, mo, :].rearrange("p (t b) -> p t b", t=T),
                        op=ALU.add)
                    hp_b = ps[:, 8, mo, :].unsqueeze(1).broadcast_to([128, T, BS])
                    nc.vector.tensor_tensor(out=pgm, in0=pgm, in1=hp_b, op=ALU.add)
                nc.scalar.activation(pgf[:, :, :], pgf[:, :, :], AF.Sigmoid)
                nc.sync.dma_start(out=opg_d[k], in_=pgf[:, :, :])

                last = (k == n_outer - 1)
                h_pre_next = outs[:, :, (T - 1) * BS:T * BS]
                if last and not carry_out:
                    h_pre = h_pre_next
                    break

                # ---- argmax via transpose + free-dim max, then value-match ----
                for tt in range(2):
                    for kk in range(NKU):
                        j = tt * 4 + kk
                        nc.tensor.transpose(
                            ps[:, 10 + j // 2, (j % 2) * 16:(j % 2) * 16 + 16, :],
                            outs[:, kk, tt * 128:(tt + 1) * 128],
                            ident[:, :],
                        )
                for tt in range(2):
                    nc.vector.tensor_reduce(
                        out=maxcol[:, tt:tt + 1],
                        in_=ps[:, 10 + 2 * tt:12 + 2 * tt, :, :],
                        axis=AX.XYZ, op=ALU.max)
                for tt in range(2):
                    nc.tensor.transpose(
                        ps[0:1, 15, tt * 16:(tt + 1) * 16, :],
                        maxcol[:, tt:tt + 1],
                        ident[:, :],
                    )
                    nc.scalar.copy(
                        maxrow[0:1, tt * 128:(tt + 1) * 128],
                        ps[0:1, 15, tt * 16:(tt + 1) * 16, :].rearrange(
                            "p t b -> p (t b)"))
                nc.tensor.matmul(
                    ps[:, 14, :, :], onesc[:, :], maxrow[:, :],
                    start=True, stop=True, skip_group_check=True)
                mxb = ps[:, 14, :, :].rearrange("p t b -> p (t b)")
                for kk in range(NKU):
                    nc.vector.tensor_tensor(
                        out=onehot[:, kk, :], in0=outs[:, kk, :],
                        in1=mxb, op=ALU.is_equal)
                # new dec_in^T = emb512^T-gather via one-hot matmul
                for ee in range(2):
                    for kk in range(NKU):
                        nc.tensor.matmul(
                            ps[:, 2 * ee, :, :],
                            mB(embl[:, kk, ee * 128:(ee + 1) * 128]),
                            mB(onehot[:, kk, :]),
                            start=(kk == 0), stop=(kk == NKU - 1),
                            skip_group_check=True,
                        )
                for ee in range(2):
                    nc.scalar.copy(dec[:, ee, :], ps[:, 2 * ee, :, :].rearrange("p t b -> p (t b)"))

                # ---- xg for k+1 ----
                if not last:
                    xg_load()

                h_pre = h_pre_next

            if carry_out:
                nc.sync.dma_start(out=ocd_d[:, :, :], in_=dec[:, :, :])
                nc.sync.dma_start(out=och_d[:, :, :], in_=h_pre)
                nc.sync.dma_start(out=occ_d[:, :, :], in_=c_prev[:, :, :])

    nc.compile()
    return nc


def _pack_inputs(inputs):
    f32 = np.float32
    emb = np.ascontiguousarray(inputs["emb"], dtype=f32)
    Wk = np.asarray(inputs["Wk"], dtype=f32)
    Wr = np.asarray(inputs["Wr"], dtype=f32)
    b = np.asarray(inputs["b"], dtype=f32)
    Wgen = np.asarray(inputs["Wgen"], dtype=f32)
    bgen = np.asarray(inputs["bgen"], dtype=f32)
    enc = np.asarray(inputs["encoder_output"], dtype=f32)
    target = np.asarray(inputs["target"])
    enc_h = np.asarray(inputs["enc_h"], dtype=f32)
    enc_c = np.asarray(inputs["enc_c"], dtype=f32)

    def chunkP(a, nk):  # [nk*128, F] -> [128, nk*F]
        return np.ascontiguousarray(
            a.reshape(nk, 128, -1).transpose(1, 0, 2).reshape(128, -1))

    def put(blob, name, arr):
        o, n = BLOB_OFF[name]
        assert arr.shape == (128, n), (name, arr.shape, n)
        blob[:, o:o + n] = arr

    shared = np.zeros((128, BLOB_N), f32)
    wf = np.zeros((128, 7, G), f32)
    wf[:, 0:4] = chunkP(Wr, 4).reshape(128, 4, G)
    wf[:, 4:6] = chunkP(Wk, 2).reshape(128, 2, G)
    wf[0, 6, :] = b
    put(shared, "wf", wf.reshape(128, -1))
    put(shared, "embl", chunkP(emb[:512], 4))
    put(shared, "wgh", chunkP(Wgen[512:1024], 4))
    wgd = np.zeros((128, 3, U), f32)
    wgd[:, 0:2] = chunkP(Wgen[1024:1280], 2).reshape(128, 2, U)
    wgd[0, 2, :] = bgen
    put(shared, "wgd", wgd.reshape(128, -1))
    put(shared, "wge", chunkP(Wgen[0:512], 4))
    put(shared, "ident", np.eye(128, dtype=f32))
    onesc = np.zeros((128, 128), f32)
    onesc[0, :] = 1.0
    put(shared, "onesc", onesc)

    maps = []
    for c in range(NC):
        blob = shared.copy()
        sl = slice(c * BS, (c + 1) * BS)
        d0 = emb[target[sl]]                        # [BS, T, E]
        d0T = d0.transpose(2, 1, 0).reshape(E, TB)  # [E, (t,b)]
        dec0 = np.zeros((128, 3, TB), f32)
        dec0[:, 0:2] = chunkP(d0T, 2).reshape(128, 2, TB)
        dec0[0, 2, :] = 1.0
        put(blob, "dec", dec0.reshape(128, -1))
        put(blob, "encT", chunkP(enc[sl].transpose(2, 1, 0).reshape(U, TB), 4))
        put(blob, "h0", chunkP(enc_h[sl].T.copy(), 4))
        put(blob, "c0", chunkP(enc_c[sl].T.copy(), 4))
        maps.append({"blob": blob})
    return maps


def _unpack(results, n_outer=KO):
    dec_outs = np.empty((n_outer, B, T, U), np.float32)
    p_gens = np.empty((n_outer, B, T, U), np.float32)
    hs = np.empty((n_outer, B, U), np.float32)
    cs = np.empty((n_outer, B, U), np.float32)
    for c, r in enumerate(results):
        sl = slice(c * BS, (c + 1) * BS)
        for name, dst in (("odo", dec_outs), ("opg", p_gens)):
            a = r[name].reshape(n_outer, 128, NKU, T, BS)
            dst[:, sl] = a.transpose(0, 4, 3, 2, 1).reshape(n_outer, BS, T, U)
        for name, dst in (("ohs", hs), ("ocs", cs)):
            a = r[name].reshape(n_outer, 128, NKU, BS)
            dst[:, sl] = a.transpose(0, 3, 2, 1).reshape(n_outer, BS, U)
    return dec_outs, p_gens, hs, cs


_RUN_KW = {}
LAST = None


def kernel(input_seq, target, encoder_output, enc_h, enc_c, emb, Wk, Wr, b,
           Wgen, bgen, n_outer=KO, rec_r=False, big_r=False):
    global LAST
    inputs = dict(target=target, encoder_output=encoder_output, enc_h=enc_h,
                  enc_c=enc_c, emb=emb, Wk=Wk, Wr=Wr, b=b, Wgen=Wgen, bgen=bgen)
    maps = _pack_inputs(inputs)
    # Fully-unrolled 31-step program overflows the 16-bit engine semaphore
    # counters (~66K PE ticks); split into segments and carry (dec, h, c).
    segs = []
    rem = n_outer
    while rem > 0:
        s = min(rem, SEG_MAX)
        rem -= s
        segs.append((s, rem > 0))
    all_res = []
    for s, carry in segs:
        nc = build(n_outer=s, rec_r=rec_r, big_r=big_r, carry_out=carry)
        res = run_bass_kernel_spmd(nc, maps, list(range(NC)), **_RUN_KW)
        LAST = res
        all_res.append(res.results)
        if carry:
            maps = [dict(m) for m in maps]
            for c, (m, r) in enumerate(zip(maps, res.results)):
                blob = m["blob"].copy()
                for nm, key in (("dec", "ocd"), ("h0", "och"), ("c0", "occ")):
                    o, n = BLOB_OFF[nm]
                    blob[:, o:o + n] = r[key].reshape(128, n)
                m["blob"] = blob
    parts = [_unpack(r, n_outer=s) for r, (s, _) in zip(all_res, segs)]
    return tuple(np.concatenate([p[i] for p in parts], axis=0) for i in range(4))
